# revision 31
# baseline (speedup 1.0000x reference)
"""Trainium2 Bass kernel for DecoderAttn ('general' attention score + softmax).

Reference computation (per batch b):
    energies[t] = dec[b] . (enc[b,t] @ W.T + bias)
    attn = softmax(energies over t)

Algebraic rewrite used here:
    energies[t] = enc[b,t] . (W.T @ dec[b])  +  (bias . dec[b])
The second term is constant over t, so it drops out of the softmax exactly.
This turns an O(B*T*H^2) matmul chain into an O(B*H^2 + B*T*H) streaming
problem that is HBM-bandwidth bound on the encoder stream.

Sharding: data-parallel over batch B=32 across 8 NeuronCores (4 batches per
core), W replicated (hint-compliant).

The default MODE="pe" kernel (build_kernel_pe):
- host casts everything to fp16 (rel_err ~1.7e-3, gate is 2e-2) and
  pre-transposes the encoder to [bl, H, T], halving HBM traffic per core
  from 32MB to 16.8MB and putting the contraction dim on partitions;
- v = dec @ W is computed on the tensor engine at startup and transposed
  to vT [128(h), bl] fp16 columns;
- energies are tensor-engine matvecs: matmul(lhsT=vT column [128,1],
  rhs=encT tile [128,512]) accumulated over 8 h-chunks in PSUM, with
  tile_position=(0, 32*b) landing batch b on PSUM partition 32b so the
  softmax runs batched over all 4 batches as [128, 2048]
  partition-parallel ops (DVE max-reduce, ACT exp+sum, DVE scale);
- the DVE/ACT engines touch only the tiny softmax, so the encoder DMA
  stream feeds the otherwise-idle PE directly and the kernel tracks the
  DMA roofline (~110us fp32 baseline -> ~60us).

The older MODE="dve" path (build_kernel) streams enc in the natural
[bl, T, H] layout and does DVE tensor_mul + ACT accumulate per tile, with
optional 8-way W row-shard + ReduceScatter (WSHARD).
"""

import numpy as np
from contextlib import ExitStack

import concourse.bass as bass
import concourse.tile as tile
from concourse import bacc, mybir, masks
from concourse.bass_utils import run_bass_kernel_spmd

F32 = mybir.dt.float32
F16 = mybir.dt.float16

B, T, H = 32, 2048, 1024
NCORES = 8
BL = B // NCORES           # batches per core
TCH = T // 128             # 128-row t-chunks per batch
OCH = H // 128             # 128-row o-chunks of W


def build_kernel(bl=BL, t=T, h=H, enc_bufs=24, repeat=1, scr_bufs=4, pair=False,
                 wshard=False, n_cores=NCORES, startup_in_loop=False,
                 startup_dma="scalar", f16=False):
    tch = t // 128
    och = h // 128
    nhh = h // 512  # 512-wide halves of the H free dim for matmul N-limit
    EDT = F16 if f16 else F32  # dtype of the streamed encoder + broadcast v

    nc = bacc.Bacc("TRN2", target_bir_lowering=False, debug=False)

    if wshard:
        # every core gets: dec columns for ITS o-chunk [B, 128], W rows for
        # ITS o-chunk [128, h]; partial v is summed across cores with a
        # ReduceScatter that hands core c exactly its 4 batches' v.
        nb = bl * n_cores
        dec = nc.dram_tensor("dec", [nb, 128], F32, kind="ExternalInput")
        w = nc.dram_tensor("w", [128, h], F32, kind="ExternalInput")
    else:
        dec = nc.dram_tensor("dec", [bl, h], F32, kind="ExternalInput")
        w = nc.dram_tensor("w", [h, h], F32, kind="ExternalInput")
    enc = nc.dram_tensor("enc", [bl, t, h], EDT, kind="ExternalInput")
    attn = nc.dram_tensor("attn", [bl, t], F32, kind="ExternalOutput")

    with tile.TileContext(nc) as tc, ExitStack() as ctx:
        const = ctx.enter_context(tc.tile_pool(name="const", bufs=1))
        wpool = ctx.enter_context(tc.tile_pool(name="wpool", bufs=1))
        encp = ctx.enter_context(tc.tile_pool(name="encp", bufs=enc_bufs))
        scr = ctx.enter_context(tc.tile_pool(name="scr", bufs=scr_bufs))
        sm = ctx.enter_context(tc.tile_pool(name="sm", bufs=2))
        outp = ctx.enter_context(tc.tile_pool(name="outp", bufs=2))
        psA = ctx.enter_context(tc.tile_pool(name="psA", bufs=2, space="PSUM"))
        psS = ctx.enter_context(tc.tile_pool(name="psS", bufs=3, space="PSUM"))

        sdma = getattr(nc, startup_dma)

        # ---- constants ----
        ident = const.tile([128, 128], F32)
        masks.make_identity(nc, ident[:])
        ones = const.tile([1, 128], F32)
        nc.gpsimd.memset(ones[:], 1.0)

        # long-lived state
        epool = ctx.enter_context(tc.tile_pool(name="epool", bufs=2))
        vrep = 2 if pair else 1
        vb_all = const.tile([128, bl * vrep * h], EDT)  # v[b] bcast (x2 if paired)

        def do_startup(rep):
            v_sb = const.tile([1, bl * h], F32, tag="v_sb")  # rows on partition 0
            if wshard:
                # phase 1 (sharded W): partial v over this core's o-chunk,
                # ReduceScatter-add across cores
                dec_sb = const.tile([nb, 128], F32, tag="dec_sb")
                decT = const.tile([128, nb], F32, tag="decT")
                pv_sb = const.tile([nb, h], F32, tag="pv_sb")
                cc_in = nc.dram_tensor(f"cc_in{rep}", [nb, h], F32)
                cc_out = nc.dram_tensor(f"cc_out{rep}", [bl, h], F32)

                sdma.dma_start(dec_sb[:], dec[:, :])
                dT_ps = psS.tile([128, nb], F32, tag="small")
                nc.tensor.transpose(dT_ps[:], dec_sb[:, :], ident[0:nb, 0:nb])
                nc.vector.tensor_copy(decT[:, :], dT_ps[:])
                wt = wpool.tile([128, h], F32, tag="w0")
                sdma.dma_start(wt[:], w[:, :])
                for hh in range(nhh):
                    pv_ps = psA.tile([nb, 512], F32, tag="work")
                    nc.tensor.matmul(
                        pv_ps[:], decT[:, :], wt[:, hh * 512:(hh + 1) * 512],
                        start=True, stop=True,
                    )
                    nc.vector.tensor_copy(
                        pv_sb[:, hh * 512:(hh + 1) * 512], pv_ps[:]
                    )
                sdma.dma_start(cc_in[:, :], pv_sb[:])
                nc.gpsimd.collective_compute(
                    "ReduceScatter",
                    mybir.AluOpType.add,
                    replica_groups=[list(range(n_cores))],
                    ins=[cc_in[:]],
                    outs=[cc_out[:]],
                )
                sdma.dma_start(
                    v_sb[0:1, :],
                    cc_out[:, :].rearrange("(one a) b -> one (a b)", one=1),
                )
            else:
                # phase 1 (replicated W): v = dec @ W on this core
                dec_sb = const.tile([bl, h], F32, tag="dec_sb")
                decT = const.tile([128, och * bl], F32, tag="decT")
                sdma.dma_start(dec_sb[:], dec[:, :])

                for oc in range(och):
                    dT_ps = psS.tile([128, bl], F32, tag="small")
                    nc.tensor.transpose(
                        dT_ps[:], dec_sb[:, oc * 128:(oc + 1) * 128],
                        ident[0:bl, 0:bl]
                    )
                    nc.vector.tensor_copy(
                        decT[:, oc * bl:(oc + 1) * bl], dT_ps[:]
                    )

                w_tiles = []
                for oc in range(och):
                    wt = wpool.tile([128, h], F32, tag=f"w{oc}")
                    sdma.dma_start(wt[:], w[oc * 128:(oc + 1) * 128, :])
                    w_tiles.append(wt)

                for b in range(bl):
                    for hh in range(nhh):
                        v_ps = psA.tile([1, 512], F32, tag="work")
                        for oc in range(och):
                            nc.tensor.matmul(
                                v_ps[:],
                                decT[:, oc * bl + b: oc * bl + b + 1],
                                w_tiles[oc][:, hh * 512:(hh + 1) * 512],
                                start=(oc == 0),
                                stop=(oc == och - 1),
                            )
                        nc.vector.tensor_copy(
                            v_sb[:, b * h + hh * 512: b * h + (hh + 1) * 512],
                            v_ps[:]
                        )

            # phase 2: broadcast v[b] across all 128 partitions
            for b in range(bl):
                for hh in range(nhh):
                    vb_ps = psA.tile([128, 512], F32, tag="work")
                    nc.tensor.matmul(
                        vb_ps[:],
                        ones[0:1, 0:128],
                        v_sb[0:1, b * h + hh * 512: b * h + (hh + 1) * 512],
                        start=True,
                        stop=True,
                    )
                    for rr in range(vrep):
                        nc.scalar.copy(
                            vb_all[:, (b * vrep + rr) * h + hh * 512:
                                   (b * vrep + rr) * h + (hh + 1) * 512], vb_ps[:]
                        )

        # ---- phase 3+4: stream encoder, fused dot, softmax ----
        if not startup_in_loop:
            do_startup(0)
        for _rep in range(repeat):
            if startup_in_loop:
                do_startup(_rep)
            if pair:
                _phase34_pair(nc, tc, bl, t, h, tch, enc, attn, encp, scr, sm,
                              outp, psS, epool, vb_all, ones, ident, EDT)
            else:
                _phase34(nc, tc, bl, t, h, tch, enc, attn, encp, scr, sm, outp,
                         psS, epool, vb_all, ones, ident, EDT)

    nc.compile()
    return nc


def _phase34(nc, tc, bl, t, h, tch, enc, attn, encp, scr, sm, outp, psS,
             epool, vb_all, ones, ident, edt=F32):
    if True:
        for b in range(bl):
            vb = vb_all[:, b * h:(b + 1) * h]
            e_t = epool.tile([128, tch], F32, tag=f"e{b}")
            for tcix in range(tch):
                et = encp.tile([128, h], edt, tag="enc")
                nc.sync.dma_start(et[:], enc[b, tcix * 128:(tcix + 1) * 128, :])
                sc = scr.tile([128, h], edt, tag="scr")
                nc.vector.tensor_mul(sc[:], et[:], vb)
                dump = scr.tile([128, h], edt, tag="dump")
                nc.scalar.activation(
                    dump[:], sc[:], mybir.ActivationFunctionType.Copy,
                    bias=0.0, scale=1.0,
                    accum_out=e_t[:, tcix: tcix + 1],
                )

            _softmax_batch(nc, b, tch, attn, sm, outp, psS, e_t, ones, ident)


def _phase34_pair(nc, tc, bl, t, h, tch, enc, attn, encp, scr, sm, outp, psS,
                  epool, vb_all, ones, ident, edt=F32):
    for b in range(bl):
        vb2 = vb_all[:, b * 2 * h:(b + 1) * 2 * h]
        e_t = epool.tile([128, tch], F32, tag=f"e{b}")
        for tp in range(tch // 2):
            et = encp.tile([128, 2 * h], edt, tag="enc")
            nc.sync.dma_start(
                et[:].rearrange("p (n h) -> p n h", n=2),
                enc[b, tp * 256:(tp + 1) * 256, :].rearrange(
                    "(n p) h -> p n h", p=128),
            )
            sc = scr.tile([128, 2 * h], edt, tag="scr")
            nc.vector.tensor_mul(sc[:], et[:], vb2)
            for n in range(2):
                dump = scr.tile([128, h], edt, tag="dump")
                nc.scalar.activation(
                    dump[:], sc[:, n * h:(n + 1) * h],
                    mybir.ActivationFunctionType.Copy,
                    bias=0.0, scale=1.0,
                    accum_out=e_t[:, 2 * tp + n: 2 * tp + n + 1],
                )
        _softmax_batch(nc, b, tch, attn, sm, outp, psS, e_t, ones, ident)


def _softmax_batch(nc, b, tch, attn, sm, outp, psS, e_t, ones, ident):
    if True:
        if True:
            # softmax over the [128, tch] energies of this batch
            e_b = e_t[:, :]

            m1 = sm.tile([128, 1], F32, tag="m1")
            nc.vector.tensor_reduce(
                out=m1[:], in_=e_b, axis=mybir.AxisListType.X, op=mybir.AluOpType.max
            )
            m1T = psS.tile([1, 128], F32, tag="small")
            nc.tensor.transpose(m1T[:], m1[:], ident[:, :])
            M = sm.tile([1, 1], F32, tag="M")
            nc.vector.tensor_reduce(
                out=M[:], in_=m1T[0:1, :], axis=mybir.AxisListType.X,
                op=mybir.AluOpType.max,
            )
            Mb_ps = psS.tile([128, 1], F32, tag="small")
            nc.tensor.matmul(Mb_ps[:], ones[0:1, 0:128], M[0:1, 0:1],
                             start=True, stop=True)
            negM = sm.tile([128, 1], F32, tag="negM")
            nc.scalar.mul(negM[:], Mb_ps[:], -1.0)

            p_b = sm.tile([128, tch], F32, tag="p")
            s1 = sm.tile([128, 1], F32, tag="s1")
            nc.scalar.activation(
                p_b[:], e_b, mybir.ActivationFunctionType.Exp,
                bias=negM[:, 0:1], scale=1.0, accum_out=s1[:],
            )
            s1T = psS.tile([1, 128], F32, tag="small")
            nc.tensor.transpose(s1T[:], s1[:], ident[:, :])
            S = sm.tile([1, 1], F32, tag="S")
            nc.vector.tensor_reduce(
                out=S[:], in_=s1T[0:1, :], axis=mybir.AxisListType.X,
                op=mybir.AluOpType.add,
            )
            R = sm.tile([1, 1], F32, tag="R")
            nc.vector.reciprocal(R[:], S[:])
            Rb_ps = psS.tile([128, 1], F32, tag="small")
            nc.tensor.matmul(Rb_ps[:], ones[0:1, 0:128], R[0:1, 0:1],
                             start=True, stop=True)
            Rb = sm.tile([128, 1], F32, tag="Rbs")
            nc.scalar.copy(Rb[:], Rb_ps[:])

            a_b = sm.tile([128, tch], F32, tag="a")
            nc.vector.tensor_scalar_mul(a_b[:], p_b[:], Rb[:, 0:1])

            aT_ps = psS.tile([tch, 128], F32, tag="small")
            nc.tensor.transpose(aT_ps[:], a_b[:], ident[:, :])
            aT = outp.tile([tch, 128], F32, tag="aTs")
            nc.vector.tensor_copy(aT[:], aT_ps[:])
            nc.sync.dma_start(
                attn[b].rearrange("(c p) -> c p", p=128), aT[:]
            )


def build_kernel_pe(bl=BL, t=T, h=H, enc_bufs=8, repeat=1, n_cores=NCORES,
                    startup_in_loop=False, startup_dma="scalar", hw_loop=1,
                    wshard=False, dual_dma=False, pair_dma=False,
                    tri_dma=False):
    """PE-path variant: host supplies encT [bl, H, T] fp16 (pre-transposed),
    so the energies are straight tensor-engine matvecs
        e[b, t] = sum_h encT[b, h, t] * v[b, h]
    with the contraction dim on partitions.  Each batch lands on PSUM
    partition 32*b (tile_position col groups), so the softmax runs batched
    over all bl batches as plain [128, T] partition-parallel ops.
    DVE/ACT do softmax only; the encoder stream feeds the PE directly.

    wshard: each core holds one 128-row o-slice of W and the matching dec
    columns for ALL batches; a ReduceScatter-add of the partial v hands
    core c its own bl batches' v (cuts W DMA 2MB -> 0.25MB per core).
    """
    hch = h // 128            # 128-row h-chunks
    nnb = t // 512            # 512-wide N blocks (PSUM bank limit, f32)
    nb_all = bl * n_cores

    nc = bacc.Bacc("TRN2", target_bir_lowering=False, debug=False)

    if wshard:
        dec = nc.dram_tensor("dec", [nb_all, 128], F16, kind="ExternalInput")
        w = nc.dram_tensor("w", [128, h], F16, kind="ExternalInput")
    else:
        dec = nc.dram_tensor("dec", [bl, h], F16, kind="ExternalInput")
        w = nc.dram_tensor("w", [h, h], F16, kind="ExternalInput")
    enc = nc.dram_tensor("enc", [bl, h, t], F16, kind="ExternalInput")
    attn = nc.dram_tensor("attn", [bl, t], F32, kind="ExternalOutput")

    with tile.TileContext(nc) as tc, ExitStack() as ctx:
        const = ctx.enter_context(tc.tile_pool(name="const", bufs=1))
        # startup-chain tiles double-buffered so rep r's v-chain overlaps
        # rep r-1's encoder streaming in the unrolled bench loop
        vp = ctx.enter_context(tc.tile_pool(name="vp", bufs=2))
        wpool = ctx.enter_context(tc.tile_pool(name="wpool", bufs=2))
        encp = ctx.enter_context(tc.tile_pool(name="encp", bufs=enc_bufs))
        sm = ctx.enter_context(tc.tile_pool(name="sm", bufs=2))
        psE = ctx.enter_context(tc.tile_pool(name="psE", bufs=1, space="PSUM"))
        psA = ctx.enter_context(tc.tile_pool(name="psA", bufs=2, space="PSUM"))
        psS = ctx.enter_context(tc.tile_pool(name="psS", bufs=1, space="PSUM"))

        sdma = getattr(nc, startup_dma)

        ident16 = const.tile([128, 128], F16)
        masks.make_identity(nc, ident16[:])
        ident32 = const.tile([128, 128], F32)
        masks.make_identity(nc, ident32[:])

        def do_startup_wshard(rep):
            # this core's o-slice: dec [nb_all, 128] f16, w [128, h] f16
            dec_sb = vp.tile([nb_all, 128], F16, tag="dec_sb")
            sdma.dma_start(dec_sb[:], dec[:, :])
            decT = vp.tile([128, nb_all], F16, tag="decT")
            dT_ps = psS.tile([128, nb_all], F16, tag="small16")
            nc.tensor.transpose(dT_ps[:], dec_sb[:, :],
                                ident16[0:nb_all, 0:nb_all])
            nc.vector.tensor_copy(decT[:, :], dT_ps[:])
            wt = wpool.tile([128, h], F16, tag="w0")
            sdma.dma_start(wt[:], w[:, :])

            # partial v over this o-slice, all batches: [nb_all, h]
            pv_sb = vp.tile([nb_all, h], F32, tag="pv_sb")
            for hh in range(h // 512):
                v_ps = psA.tile([nb_all, 512], F32, tag="vps")
                nc.tensor.matmul(
                    v_ps[:], decT[:, :], wt[:, hh * 512:(hh + 1) * 512],
                    start=True, stop=True)
                nc.vector.tensor_copy(
                    pv_sb[:, hh * 512:(hh + 1) * 512], v_ps[:])

            cc_in = nc.dram_tensor(f"cc_in{rep}", [nb_all, h], F32)
            cc_out = nc.dram_tensor(f"cc_out{rep}", [bl, h], F32)
            sdma.dma_start(cc_in[:, :], pv_sb[:])
            nc.gpsimd.collective_compute(
                "ReduceScatter", mybir.AluOpType.add,
                replica_groups=[list(range(n_cores))],
                ins=[cc_in[:]], outs=[cc_out[:]])
            v_sb = vp.tile([bl, h], F32, tag="v_sb")
            sdma.dma_start(v_sb[:], cc_out[:, :])

            # transpose v to vT [128(h), hc*bl] fp16
            vT = vp.tile([128, hch * bl], F16, tag="vT")
            for hc in range(hch):
                vt_ps = psS.tile([128, bl], F32, tag="small")
                nc.tensor.transpose(
                    vt_ps[:], v_sb[:, hc * 128:(hc + 1) * 128],
                    ident32[0:bl, 0:bl])
                nc.vector.tensor_copy(vT[:, hc * bl:(hc + 1) * bl], vt_ps[:])
            return vT

        def do_startup_repl(rep):
            # dec [bl, h] fp16 -> decT chunks [128, bl] per o-chunk
            dec_sb = vp.tile([bl, h], F16, tag="dec_sb")
            sdma.dma_start(dec_sb[:], dec[:, :])
            decT = vp.tile([128, hch * bl], F16, tag="decT")
            for oc in range(hch):
                dT_ps = psS.tile([128, bl], F16, tag="small16")
                nc.tensor.transpose(
                    dT_ps[:], dec_sb[:, oc * 128:(oc + 1) * 128],
                    ident16[0:bl, 0:bl])
                nc.vector.tensor_copy(decT[:, oc * bl:(oc + 1) * bl], dT_ps[:])

            w_tiles = []
            for oc in range(hch):
                wt = wpool.tile([128, h], F16, tag=f"w{oc}")
                sdma.dma_start(wt[:], w[oc * 128:(oc + 1) * 128, :])
                w_tiles.append(wt)

            # v[b, hh-block] = sum_oc decT[oc].T @ W[oc, hh-block]
            v_sb = vp.tile([bl, h], F32, tag="v_sb")
            for hh in range(h // 512):
                v_ps = psA.tile([bl, 512], F32, tag="vps")
                for oc in range(hch):
                    nc.tensor.matmul(
                        v_ps[:], decT[:, oc * bl:(oc + 1) * bl],
                        w_tiles[oc][:, hh * 512:(hh + 1) * 512],
                        start=(oc == 0), stop=(oc == hch - 1))
                nc.vector.tensor_copy(
                    v_sb[:, hh * 512:(hh + 1) * 512], v_ps[:])

            # transpose v to vT [128(h), hc*bl] fp16
            vT = vp.tile([128, hch * bl], F16, tag="vT")
            for hc in range(hch):
                vt_ps = psS.tile([128, bl], F32, tag="small")
                nc.tensor.transpose(
                    vt_ps[:], v_sb[:, hc * 128:(hc + 1) * 128],
                    ident32[0:bl, 0:bl])
                nc.vector.tensor_copy(vT[:, hc * bl:(hc + 1) * bl], vt_ps[:])
            return vT

        do_startup = do_startup_wshard if wshard else do_startup_repl

        # energies accumulator: allocated once; rows not hit by any matmul
        # are zero-filled once so softmax lanes stay finite
        eps = psE.tile([128, t], F32, tag="eps")
        nc.vector.memset(eps[:], 0.0)

        def workload(_rep, vT):
            # ---- stream encT, energies via PE matvec ----
            if tri_dma:
                # 2 HWDGE rings + the SWDGE (gpsimd/Q7) ring
                dma_engines = (nc.sync, nc.scalar, nc.gpsimd)
            elif dual_dma:
                dma_engines = (nc.sync, nc.scalar)
            else:
                dma_engines = (nc.sync,)
            step = 2 if pair_dma else 1
            for b in range(bl):
                for hci, hc in enumerate(range(0, hch, step)):
                    eng = dma_engines[hci % len(dma_engines)]
                    et = encp.tile([128, step * t], F16, tag="enc")
                    if pair_dma:
                        eng.dma_start(
                            et[:].rearrange("p (n t) -> p n t", n=step),
                            enc[b, hc * 128:(hc + step) * 128, :].rearrange(
                                "(n p) t -> p n t", p=128))
                    else:
                        eng.dma_start(et[:],
                                      enc[b, hc * 128:(hc + 1) * 128, :])
                    for n in range(step):
                        for nb in range(nnb):
                            nc.tensor.matmul(
                                eps[32 * b:32 * b + 1,
                                    nb * 512:(nb + 1) * 512],
                                vT[:, (hc + n) * bl + b:(hc + n) * bl + b + 1],
                                et[:, n * t + nb * 512:n * t + (nb + 1) * 512],
                                start=(hc + n == 0), stop=(hc + n == hch - 1),
                                tile_position=(0, 32 * b))

            # ---- batched softmax over the free dim, rows 32b ----
            m = sm.tile([128, 1], F32, tag="m")
            nc.vector.tensor_reduce(
                out=m[:], in_=eps[:, :], axis=mybir.AxisListType.X,
                op=mybir.AluOpType.max)
            negM = sm.tile([128, 1], F32, tag="negM")
            nc.scalar.mul(negM[:], m[:], -1.0)
            p = sm.tile([128, t], F32, tag="p")
            s = sm.tile([128, 1], F32, tag="s")
            nc.scalar.activation(
                p[:], eps[:, :], mybir.ActivationFunctionType.Exp,
                bias=negM[:, 0:1], scale=1.0, accum_out=s[:])
            r = sm.tile([128, 1], F32, tag="r")
            nc.vector.reciprocal(r[:], s[:])
            a = sm.tile([128, t], F32, tag="a")
            nc.vector.tensor_scalar_mul(a[:], p[:], r[:, 0:1])
            for b in range(bl):
                nc.sync.dma_start(attn[b:b + 1, :], a[32 * b:32 * b + 1, :])

        def body():
            vT0 = None
            if not startup_in_loop:
                vT0 = do_startup(0)
            for _rep in range(repeat):
                vT = do_startup(_rep) if startup_in_loop else vT0
                workload(_rep, vT)

        if hw_loop > 1:
            # the unrolled body far exceeds one 16KiB IRAM block per engine,
            # so arm the branch prefetcher to avoid ~4us back-edge I$ stalls
            hints = (mybir.EngineType.PE, mybir.EngineType.SP,
                     mybir.EngineType.DVE, mybir.EngineType.Activation)
            with tc.For_i(0, hw_loop, 1, name="bench_loop",
                          hint_engines=hints):
                body()
        else:
            body()

    nc.compile()
    return nc


def shard_inputs_pe(decoder_output, encoder_outputs, W, wshard=False):
    """Per-core inputs for the PE path: encT [bl, H, T] fp16, dec/W fp16."""
    enc16 = encoder_outputs.astype(np.float16)
    maps = []
    for c in range(NCORES):
        sl = slice(c * BL, (c + 1) * BL)
        m = {"enc": np.ascontiguousarray(enc16[sl].transpose(0, 2, 1))}
        if wshard:
            m["dec"] = np.ascontiguousarray(
                decoder_output[:, c * 128:(c + 1) * 128], dtype=np.float16)
            m["w"] = np.ascontiguousarray(
                W[c * 128:(c + 1) * 128, :], dtype=np.float16)
        else:
            m["dec"] = np.ascontiguousarray(decoder_output[sl],
                                            dtype=np.float16)
            m["w"] = np.ascontiguousarray(W, dtype=np.float16)
        maps.append(m)
    return maps


_NC_CACHE = {}


WSHARD = True  # shard W 8-ways + ReduceScatter partial v (saves 3.5MB/core DMA)
USE_F16 = True  # stream the encoder in fp16 (host-side cast halves HBM traffic)
MODE = "pe"    # "pe": tensor-engine matvec on host-transposed enc; "dve": mul+accum


def _get_nc():
    if "nc" not in _NC_CACHE:
        if MODE == "pe":
            _NC_CACHE["nc"] = build_kernel_pe(enc_bufs=12, dual_dma=True)
        else:
            _NC_CACHE["nc"] = build_kernel(wshard=WSHARD, f16=USE_F16)
    return _NC_CACHE["nc"]


def shard_inputs(decoder_output, encoder_outputs, W, wshard=False, f16=False):
    """Per-core input dicts for the chosen W distribution scheme."""
    edt = np.float16 if f16 else np.float32
    maps = []
    for c in range(NCORES):
        sl = slice(c * BL, (c + 1) * BL)
        m = {"enc": np.ascontiguousarray(encoder_outputs[sl], dtype=edt)}
        if wshard:
            m["dec"] = np.ascontiguousarray(
                decoder_output[:, c * 128:(c + 1) * 128], dtype=np.float32)
            m["w"] = np.ascontiguousarray(
                W[c * 128:(c + 1) * 128, :], dtype=np.float32)
        else:
            m["dec"] = np.ascontiguousarray(decoder_output[sl], dtype=np.float32)
            m["w"] = np.ascontiguousarray(W, dtype=np.float32)
        maps.append(m)
    return maps


def nc_is_wshard(nc):
    for alloc in nc.m.functions[0].allocations:
        if isinstance(alloc, mybir.MemoryLocationSet) and \
                alloc.memorylocations[0].name == "w":
            return tuple(alloc.tensor_shape) == (128, H)
    return False


def nc_is_f16(nc):
    for alloc in nc.m.functions[0].allocations:
        if isinstance(alloc, mybir.MemoryLocationSet) and \
                alloc.memorylocations[0].name == "enc":
            return alloc.dtype == mybir.dt.float16
    return False


def nc_is_pe(nc):
    # the PE path's enc layout is [bl, H, T]; the DVE path's is [bl, T, H]
    for alloc in nc.m.functions[0].allocations:
        if isinstance(alloc, mybir.MemoryLocationSet) and \
                alloc.memorylocations[0].name == "enc":
            return tuple(alloc.tensor_shape) == (BL, H, T)
    return False


def run_sharded(decoder_output, encoder_outputs, W, trace=False, nc=None, **kw):
    if nc is None:
        nc = _get_nc()
    if nc_is_pe(nc):
        in_maps = shard_inputs_pe(decoder_output, encoder_outputs, W,
                                  wshard=nc_is_wshard(nc))
    else:
        in_maps = shard_inputs(decoder_output, encoder_outputs, W,
                               wshard=nc_is_wshard(nc), f16=nc_is_f16(nc))
    res = run_bass_kernel_spmd(nc, in_maps, list(range(NCORES)), trace=trace, **kw)
    attn = np.concatenate([res.results[c]["attn"] for c in range(NCORES)], axis=0)
    return attn, res


def kernel(decoder_output, encoder_outputs, W, b=None, **_unused):
    # b (the Linear bias) shifts every energy of a batch equally -> cancels in
    # softmax; it is deliberately unused.
    attn, _ = run_sharded(decoder_output, encoder_outputs, W)
    return attn.reshape(B, T, 1).astype(np.float32)

